# revision 9
# baseline (speedup 1.0000x reference)
"""Trainium2 Bass kernel for the AllGroupsExpertRunner MoE problem.

Math (dense-masked reference):
    x = tokens.reshape(M, D)                                # M = B*N = 8192
    out = sum_e w[:, e] * (gelu(x @ Wg[e]) * (x @ Wv[e])) @ Wo[e] * scales[e]
    where w = where(dispatch > 0, combine, 0)

Sharding: expert-parallel with overflow balancing. Core e runs expert e on
up to CA of its routed tokens (weight set A); tokens beyond CA ("overflow"
of hot experts) are donated to other cores' fixed-size CB token slot, which
runs against a second weight set B. The SPMD program is identical on all
cores: [CA/512 chunks of 512 @ set A] + [one CB chunk @ set B]; cores whose
B slot is unused get zero tokens/weights-copy there. The planner picks the
smallest feasible (CA, CB). For the top-2 routing here that is (2048, 128):
T=2176 slots/core vs 2304 with plain per-expert padding.

Per-core kernel (all matmul operands bf16, fp32 PSUM, rel err ~4e-3):
  stage A (per chunk, per 128-wide H block):
      g^T = Wg_blk^T @ xT-chunk   (4 accumulating matmuls over D)
      v^T = Wv_blk^T @ xT-chunk
      hT_blk = gelu(g^T) * v^T    (ACT + DVE, bf16)
  stage B (per 128-token tile): out = hT^T @ Wo (16 matmuls over H),
      scaled per-token by the routing weight, stored bf16.

Startup choreography: the first two chunks' stage A is interleaved in two
half-H phases so the first ~30us of PE work only needs the first half of
the A weights (2MB); the first matmul waits only on 128KB of x + 256KB of
weights. Weight tiles are [128, 2048] bf16 (4KB DMA rows) split across the
two HWDGE rings in demand order; set B rides behind set A.
"""

import numpy as np
import ml_dtypes

D = 512
H = 2048
E = 8
P = 128
MT = 512  # max token chunk
ND = D // P  # 4 k-tiles over D
NH = H // P  # 16 k-tiles over H

BF16 = ml_dtypes.bfloat16

_CACHE: dict = {}


def _plan(counts, n_cores):
    """Pick (CA, CB): per-core primary capacity and overflow-slot size."""
    maxc = max(counts)
    best = None
    ca_max = ((maxc + 511) // 512) * 512
    for CA in range(512, ca_max + 512, 512):
        for CB in (0, 128, 256, 384, 512):
            need = [max(0, c - CA) for c in counts]
            if CB == 0:
                if any(need):
                    continue
            else:
                if max(need) > 0 and sum((n + CB - 1) // CB for n in need) > n_cores:
                    continue
            T = CA + CB
            if best is None or T < best[0] or (T == best[0] and CB < best[2]):
                best = (T, CA, CB)
    _, CA, CB = best
    return CA, CB


def _build_program(CA: int, CB: int, act_name: str = "Gelu"):
    from contextlib import ExitStack

    import concourse.bacc as bacc
    import concourse.tile as tile
    import concourse.mybir as mybir

    assert CA % 512 == 0 and CB % 128 == 0 and 0 <= CB <= 512
    T = CA + CB
    f32 = mybir.dt.float32
    BF = mybir.dt.bfloat16

    nc = bacc.Bacc("TRN2", target_bir_lowering=False, debug=False)

    # tokens, packed transposed per chunk: cols [ND*tok0 + d*mt + j] = x[tok0+j, d*128+p]
    xp = nc.dram_tensor("xp", [P, ND * T], BF, kind="ExternalInput")
    # weight pairs: wgv[q] holds H-blocks h=2q (cols 0:1024) and 2q+1
    # (cols 1024:2048); within a block [Wg d-major 512 | Wv d-major 512].
    wgva = nc.dram_tensor("wgva", [NH // 2, P, 4 * D], BF, kind="ExternalInput")
    woa = nc.dram_tensor("woa", [NH // 4, P, 4 * D], BF, kind="ExternalInput")
    if CB:
        wgvb = nc.dram_tensor("wgvb", [NH // 2, P, 4 * D], BF, kind="ExternalInput")
        wob = nc.dram_tensor("wob", [NH // 4, P, 4 * D], BF, kind="ExternalInput")
    wc = nc.dram_tensor("wc", [P, T // P], f32, kind="ExternalInput")
    out = nc.dram_tensor("out", [T, D], BF, kind="ExternalOutput")

    # chunk specs: (mt, tok0, weight set)
    specs = [(MT, i * MT, 0) for i in range(CA // MT)]
    if CB:
        specs.append((CB, CA, 1))
    gelu = getattr(mybir.ActivationFunctionType, act_name)

    with tile.TileContext(nc) as tc, ExitStack() as ctx:
        wpool = ctx.enter_context(tc.tile_pool(name="w", bufs=1))
        xpool = ctx.enter_context(tc.tile_pool(name="x", bufs=3))
        hpool = ctx.enter_context(tc.tile_pool(name="h", bufs=2))
        gpool = ctx.enter_context(tc.tile_pool(name="g", bufs=3))
        opool = ctx.enter_context(tc.tile_pool(name="o", bufs=4))
        psg = ctx.enter_context(tc.tile_pool(name="psg", bufs=2, space="PSUM"))
        psv = ctx.enter_context(tc.tile_pool(name="psv", bufs=2, space="PSUM"))
        pso = ctx.enter_context(tc.tile_pool(name="pso", bufs=2, space="PSUM"))

        wgv_t = [
            [wpool.tile([P, 4 * D], BF, tag=f"wgv{s}{q}", name=f"wgv{s}{q}") for q in range(NH // 2)]
            for s in range(2 if CB else 1)
        ]
        wo_t = [
            [wpool.tile([P, 4 * D], BF, tag=f"wo{s}{q}", name=f"wo{s}{q}") for q in range(NH // 4)]
            for s in range(2 if CB else 1)
        ]
        wc_t = wpool.tile([P, T // P], f32, tag="wc")

        # --- DMA schedule ---
        # sync ring:   x0 d-block0, A-pair0 (split h0|h1), rest of x0, A-pairs
        #              1-3, x1, [B-pairs even], then x2.. in loop order
        # scalar ring: wc, A-pairs 4-7, woa quads, [B-pairs odd, wob quads],
        #              then output stores from the loop
        xq_t = {}
        mt0 = specs[0][0]
        xq0 = xpool.tile([P, ND * mt0], BF, tag="xq", name="xq0")
        xq_t[0] = xq0
        nc.sync.dma_start(out=xq0[:, :mt0], in_=xp[:, 0:mt0])
        nc.sync.dma_start(out=wgv_t[0][0][:, : 2 * D], in_=wgva[0, :, : 2 * D])
        nc.sync.dma_start(out=xq0[:, mt0:], in_=xp[:, mt0 : ND * mt0])
        nc.sync.dma_start(out=wgv_t[0][0][:, 2 * D :], in_=wgva[0, :, 2 * D :])
        for q in (1, 2, 3):
            nc.sync.dma_start(out=wgv_t[0][q][:], in_=wgva[q])
        if len(specs) > 1:
            mt1, tok1, _ = specs[1]
            xq1 = xpool.tile([P, ND * mt1], BF, tag="xq", name="xq1")
            xq_t[1] = xq1
            nc.sync.dma_start(out=xq1[:], in_=xp[:, ND * tok1 : ND * (tok1 + mt1)])
        if CB:
            for q in (0, 2, 4, 6):
                nc.sync.dma_start(out=wgv_t[1][q][:], in_=wgvb[q])

        nc.scalar.dma_start(out=wc_t[:], in_=wc[:])
        for q in (4, 5, 6, 7):
            nc.scalar.dma_start(out=wgv_t[0][q][:], in_=wgva[q])
        for q in range(NH // 4):
            nc.scalar.dma_start(out=wo_t[0][q][:], in_=woa[q])
        if CB:
            for q in (1, 3, 5, 7):
                nc.scalar.dma_start(out=wgv_t[1][q][:], in_=wgvb[q])
            for q in range(NH // 4):
                nc.scalar.dma_start(out=wo_t[1][q][:], in_=wob[q])

        hT_t = {}

        def ensure_x(ci):
            if ci not in xq_t:
                mt, tok0, _ = specs[ci]
                xq = xpool.tile([P, ND * mt], BF, tag="xq", name=f"xq{ci}")
                xq_t[ci] = xq
                nc.sync.dma_start(out=xq[:], in_=xp[:, ND * tok0 : ND * (tok0 + mt)])
            return xq_t[ci]

        def stage_a(ci, h_lo, h_hi):
            mt, tok0, ws = specs[ci]
            xq = ensure_x(ci)
            if ci not in hT_t:
                hT_t[ci] = hpool.tile([P, NH, mt], BF, tag="hT", name=f"hT{ci}")
            hT = hT_t[ci]
            for h in range(h_lo, h_hi):
                base = (h % 2) * 2 * D
                pg = psg.tile([P, mt], f32, tag="pg")
                pv = psv.tile([P, mt], f32, tag="pv")
                wt = wgv_t[ws][h // 2]
                for d in range(ND):
                    nc.tensor.matmul(
                        out=pg[:], lhsT=wt[:, base + d * P : base + (d + 1) * P],
                        rhs=xq[:, d * mt : (d + 1) * mt],
                        start=(d == 0), stop=(d == ND - 1),
                    )
                for d in range(ND):
                    nc.tensor.matmul(
                        out=pv[:], lhsT=wt[:, base + D + d * P : base + D + (d + 1) * P],
                        rhs=xq[:, d * mt : (d + 1) * mt],
                        start=(d == 0), stop=(d == ND - 1),
                    )
                ga = gpool.tile([P, mt], BF, tag="ga")
                nc.scalar.activation(ga[:], pg[:], gelu)
                nc.vector.tensor_mul(hT[:, h, :], ga[:], pv[:])

        def stage_b(ci):
            mt, tok0, ws = specs[ci]
            hT = hT_t.pop(ci)
            for t in range(mt // P):
                po = pso.tile([P, D], f32, tag="po")
                for h in range(NH):
                    nc.tensor.matmul(
                        out=po[:], lhsT=hT[:, h, t * P : (t + 1) * P],
                        rhs=wo_t[ws][h // 4][:, (h % 4) * D : (h % 4 + 1) * D],
                        start=(h == 0), stop=(h == NH - 1),
                    )
                ob = opool.tile([P, D], BF, tag="ob")
                j = tok0 // P + t
                nc.vector.tensor_scalar_mul(ob[:], po[:], wc_t[:, j : j + 1])
                nc.scalar.dma_start(out=out[j * P : (j + 1) * P, :], in_=ob[:])

        if len(specs) >= 2:
            # warm-up: first two chunks' stage A in two half-H phases so the
            # early PE stream only demands the first half of the A weights
            stage_a(0, 0, NH // 2)
            stage_a(1, 0, NH // 2)
            stage_a(0, NH // 2, NH)
            stage_a(1, NH // 2, NH)
            stage_b(0)
            stage_b(1)
            rest = range(2, len(specs))
        else:
            stage_a(0, 0, NH)
            stage_b(0)
            rest = range(0)
        for ci in rest:
            stage_a(ci, 0, NH)
            stage_b(ci)

    nc.compile()
    return nc


def _pack_weights(Wg_e, Wv_e, Wo_e):
    wg4 = Wg_e.reshape(ND, P, H).transpose(1, 0, 2)  # (P, ND, H)
    wv4 = Wv_e.reshape(ND, P, H).transpose(1, 0, 2)
    wgv = np.empty((NH // 2, P, 4 * D), BF16)
    for h in range(NH):
        hs = slice(h * P, (h + 1) * P)
        base = (h % 2) * 2 * D
        wgv[h // 2, :, base : base + D] = wg4[:, :, hs].reshape(P, D).astype(BF16)
        wgv[h // 2, :, base + D : base + 2 * D] = wv4[:, :, hs].reshape(P, D).astype(BF16)
    wo4 = np.ascontiguousarray(
        Wo_e.reshape(NH // 4, 4, P, D).transpose(0, 2, 1, 3).reshape(NH // 4, P, 4 * D)
    ).astype(BF16)
    return wgv, wo4


def _pack_x(x_rows, T, specs):
    """xp [P, ND*T] with per-chunk transposed d-major layout."""
    xT = np.zeros((D, T), np.float32)
    xT[:, : x_rows.shape[0]] = x_rows.T
    xp = np.empty((P, ND * T), BF16)
    for mt, tok0 in specs:
        blk = xT[:, tok0 : tok0 + mt].reshape(ND, P, mt).transpose(1, 0, 2)
        xp[:, ND * tok0 : ND * (tok0 + mt)] = blk.reshape(P, ND * mt).astype(BF16)
    return xp


def kernel(tokens, dispatch_weights, combine_weights, Wg, Wv, Wo, scales):
    from concourse.bass_utils import run_bass_kernel_spmd

    B, N, d_model = tokens.shape
    M = B * N
    x = np.ascontiguousarray(tokens.reshape(M, d_model), dtype=np.float32)
    disp = np.asarray(dispatch_weights).reshape(M, E)
    comb = np.asarray(combine_weights).reshape(M, E)
    w_all = np.where(disp > 0, comb, 0.0).astype(np.float32) * np.asarray(
        scales, np.float32
    )[None, :]

    idx = [np.nonzero(w_all[:, e])[0] for e in range(E)]
    counts = [len(i) for i in idx]
    CA, CB = _plan(counts, E)
    T = CA + CB
    specs = [(MT, i * MT) for i in range(CA // MT)]
    if CB:
        specs.append((CB, CA))

    # assign overflow slices (tokens beyond CA of hot experts) to cores' B slots
    b_assign = [None] * E  # per core: (expert, global token indices)
    if CB:
        free = list(range(E))
        for e in range(E):
            over = idx[e][CA:]
            for s in range(0, len(over), CB):
                b_assign[free.pop(0)] = (e, over[s : s + CB])

    if (CA, CB) not in _CACHE:
        _CACHE[(CA, CB)] = _build_program(CA, CB)
    nc = _CACHE[(CA, CB)]

    wpacked = [_pack_weights(
        np.asarray(Wg[e], np.float32), np.asarray(Wv[e], np.float32),
        np.asarray(Wo[e], np.float32)) for e in range(E)]

    in_maps = []
    for c in range(E):
        a_idx = idx[c][:CA]
        rows = [x[a_idx]]
        wrow = [w_all[a_idx, c]]
        if CB:
            pad_a = CA - len(a_idx)
            if pad_a:
                rows.append(np.zeros((pad_a, D), np.float32))
                wrow.append(np.zeros(pad_a, np.float32))
            if b_assign[c] is not None:
                be, b_idx = b_assign[c]
                rows.append(x[b_idx])
                wrow.append(w_all[b_idx, be])
            else:
                be = c
        x_rows = np.concatenate(rows, 0)
        w_tok = np.concatenate(wrow, 0)
        wcm = np.zeros((T // P, P), np.float32)
        wcm.reshape(-1)[: len(w_tok)] = w_tok
        m = {
            "xp": _pack_x(x_rows, T, specs),
            "wgva": wpacked[c][0],
            "woa": wpacked[c][1],
            "wc": np.ascontiguousarray(wcm.T),
        }
        if CB:
            m["wgvb"] = wpacked[be][0]
            m["wob"] = wpacked[be][1]
        in_maps.append(m)

    res = run_bass_kernel_spmd(nc, in_maps, list(range(E)))

    out = np.zeros((M, d_model), np.float32)
    for c in range(E):
        o = res.results[c]["out"]
        na = len(idx[c][:CA])
        out[idx[c][:CA]] += o[:na].astype(np.float32)
        if CB and b_assign[c] is not None:
            be, b_idx = b_assign[c]
            out[b_idx] += o[CA : CA + len(b_idx)].astype(np.float32)
    return out.reshape(B, N, d_model)


# revision 11
# speedup vs baseline: 1.1466x; 1.1466x over previous
"""Trainium2 Bass kernel for the AllGroupsExpertRunner MoE problem.

Math (dense-masked reference):
    x = tokens.reshape(M, D)                                # M = B*N = 8192
    out = sum_e w[:, e] * (gelu(x @ Wg[e]) * (x @ Wv[e])) @ Wo[e] * scales[e]
    where w = where(dispatch > 0, combine, 0)

Sharding: expert-parallel with overflow balancing. Core e runs expert e on
up to CA of its routed tokens (weight set A); tokens beyond CA ("overflow"
of hot experts) are donated to other cores' fixed-size CB token slot, which
runs against a second weight set B. The SPMD program is identical on all
cores: [CA/512 chunks of 512 @ set A] + [one CB chunk @ set B]; cores whose
B slot is unused get zero tokens/weights-copy there. The planner picks the
smallest feasible (CA, CB). For the top-2 routing here that is (2048, 128):
T=2176 slots/core vs 2304 with plain per-expert padding.

Per-core kernel (all matmul operands bf16, fp32 PSUM, rel err ~4e-3):
  stage A (per chunk, per 128-wide H block):
      g^T = Wg_blk^T @ xT-chunk   (4 accumulating matmuls over D)
      v^T = Wv_blk^T @ xT-chunk
      hT_blk = gelu(g^T) * v^T    (ACT + DVE, bf16)
  stage B (per 128-token tile): out = hT^T @ Wo (16 matmuls over H),
      scaled per-token by the routing weight, stored bf16.

Startup choreography: the first two chunks' stage A is interleaved in two
half-H phases so the first ~30us of PE work only needs the first half of
the A weights (2MB); the first matmul waits only on 128KB of x + 256KB of
weights. Weight tiles are [128, 2048] bf16 (4KB DMA rows) split across the
two HWDGE rings in demand order; set B rides behind set A.
"""

import numpy as np
import ml_dtypes

D = 512
H = 2048
E = 8
P = 128
MT = 512  # max token chunk
ND = D // P  # 4 k-tiles over D
NH = H // P  # 16 k-tiles over H

BF16 = ml_dtypes.bfloat16

_CACHE: dict = {}


def _plan(counts, n_cores):
    """Pick (CA, CB): per-core primary capacity and overflow-slot size."""
    maxc = max(counts)
    best = None
    ca_max = ((maxc + 511) // 512) * 512
    for CA in range(512, ca_max + 512, 512):
        for CB in (0, 128, 256, 384, 512):
            need = [max(0, c - CA) for c in counts]
            if CB == 0:
                if any(need):
                    continue
            else:
                if max(need) > 0 and sum((n + CB - 1) // CB for n in need) > n_cores:
                    continue
            T = CA + CB
            if best is None or T < best[0] or (T == best[0] and CB < best[2]):
                best = (T, CA, CB)
    _, CA, CB = best
    return CA, CB


def _build_program(CA: int, CB: int, act_name: str = "Gelu"):
    from contextlib import ExitStack

    import concourse.bacc as bacc
    import concourse.tile as tile
    import concourse.mybir as mybir

    assert CA % 512 == 0 and CB % 128 == 0 and 0 <= CB <= 512
    T = CA + CB
    f32 = mybir.dt.float32
    BF = mybir.dt.bfloat16

    nc = bacc.Bacc("TRN2", target_bir_lowering=False, debug=False)

    # tokens, packed transposed per chunk: cols [ND*tok0 + d*mt + j] = x[tok0+j, d*128+p]
    xp = nc.dram_tensor("xp", [P, ND * T], BF, kind="ExternalInput")
    # weight pairs: wgv[q] holds H-blocks h=2q (cols 0:1024) and 2q+1
    # (cols 1024:2048); within a block [Wg d-major 512 | Wv d-major 512].
    wgva = nc.dram_tensor("wgva", [NH // 2, P, 4 * D], BF, kind="ExternalInput")
    woa = nc.dram_tensor("woa", [NH // 4, P, 4 * D], BF, kind="ExternalInput")
    if CB:
        wgvb = nc.dram_tensor("wgvb", [NH // 2, P, 4 * D], BF, kind="ExternalInput")
        wob = nc.dram_tensor("wob", [NH // 4, P, 4 * D], BF, kind="ExternalInput")
    wc = nc.dram_tensor("wc", [P, T // P], f32, kind="ExternalInput")
    out = nc.dram_tensor("out", [T, D], BF, kind="ExternalOutput")

    # chunk specs: (mt, tok0, weight set)
    specs = [(MT, i * MT, 0) for i in range(CA // MT)]
    if CB:
        specs.append((CB, CA, 1))
    gelu = getattr(mybir.ActivationFunctionType, act_name)

    with tile.TileContext(nc) as tc, ExitStack() as ctx:
        wpool = ctx.enter_context(tc.tile_pool(name="w", bufs=1))
        xpool = ctx.enter_context(tc.tile_pool(name="x", bufs=len(specs)))
        hpool = ctx.enter_context(tc.tile_pool(name="h", bufs=2))
        gpool = ctx.enter_context(tc.tile_pool(name="g", bufs=3))
        opool = ctx.enter_context(tc.tile_pool(name="o", bufs=4))
        psg = ctx.enter_context(tc.tile_pool(name="psg", bufs=2, space="PSUM"))
        psv = ctx.enter_context(tc.tile_pool(name="psv", bufs=2, space="PSUM"))
        pso = ctx.enter_context(tc.tile_pool(name="pso", bufs=2, space="PSUM"))

        wgv_t = [
            [wpool.tile([P, 4 * D], BF, tag=f"wgv{s}{q}", name=f"wgv{s}{q}") for q in range(NH // 2)]
            for s in range(2 if CB else 1)
        ]
        wo_t = [
            [wpool.tile([P, 4 * D], BF, tag=f"wo{s}{q}", name=f"wo{s}{q}") for q in range(NH // 4)]
            for s in range(2 if CB else 1)
        ]
        wc_t = wpool.tile([P, T // P], f32, tag="wc")

        # --- DMA schedule ---
        # ALL loads ride the sync HWDGE ring in demand order: the sync engine
        # has no compute, so dma_start issue blocking is free there (putting
        # prefetch issues on the scalar engine delays the gelu ACTIVATEs
        # behind them and stalls the PE). Output stores ride the scalar ring,
        # which carries nothing else. One ring sustains ~280GB/s.
        xq_t = {}

        def emit_x(ci):
            mt, tok0, _ = specs[ci]
            xq = xpool.tile([P, ND * mt], BF, tag="xq", name=f"xq{ci}")
            xq_t[ci] = xq
            nc.sync.dma_start(out=xq[:], in_=xp[:, ND * tok0 : ND * (tok0 + mt)])
            return xq

        mt0 = specs[0][0]
        xq0 = xpool.tile([P, ND * mt0], BF, tag="xq", name="xq0")
        xq_t[0] = xq0
        nc.sync.dma_start(out=xq0[:, :mt0], in_=xp[:, 0:mt0])
        nc.sync.dma_start(out=wgv_t[0][0][:, : 2 * D], in_=wgva[0, :, : 2 * D])
        nc.sync.dma_start(out=xq0[:, mt0:], in_=xp[:, mt0 : ND * mt0])
        nc.sync.dma_start(out=wgv_t[0][0][:, 2 * D :], in_=wgva[0, :, 2 * D :])
        for q in (1, 2, 3):
            nc.sync.dma_start(out=wgv_t[0][q][:], in_=wgva[q])
        if len(specs) > 1:
            emit_x(1)
        for q in (4, 5, 6, 7):
            nc.sync.dma_start(out=wgv_t[0][q][:], in_=wgva[q])
        nc.sync.dma_start(out=wc_t[:], in_=wc[:])
        for ci in range(2, len(specs)):
            emit_x(ci)
        for q in range(NH // 4):
            nc.sync.dma_start(out=wo_t[0][q][:], in_=woa[q])
        if CB:
            for q in range(NH // 2):
                nc.sync.dma_start(out=wgv_t[1][q][:], in_=wgvb[q])
            for q in range(NH // 4):
                nc.sync.dma_start(out=wo_t[1][q][:], in_=wob[q])

        hT_t = {}

        def ensure_x(ci):
            return xq_t[ci]

        def stage_a(ci, h_lo, h_hi):
            mt, tok0, ws = specs[ci]
            xq = ensure_x(ci)
            if ci not in hT_t:
                hT_t[ci] = hpool.tile([P, NH, mt], BF, tag="hT", name=f"hT{ci}")
            hT = hT_t[ci]
            for h in range(h_lo, h_hi):
                base = (h % 2) * 2 * D
                pg = psg.tile([P, mt], f32, tag="pg")
                pv = psv.tile([P, mt], f32, tag="pv")
                wt = wgv_t[ws][h // 2]
                for d in range(ND):
                    nc.tensor.matmul(
                        out=pg[:], lhsT=wt[:, base + d * P : base + (d + 1) * P],
                        rhs=xq[:, d * mt : (d + 1) * mt],
                        start=(d == 0), stop=(d == ND - 1),
                    )
                for d in range(ND):
                    nc.tensor.matmul(
                        out=pv[:], lhsT=wt[:, base + D + d * P : base + D + (d + 1) * P],
                        rhs=xq[:, d * mt : (d + 1) * mt],
                        start=(d == 0), stop=(d == ND - 1),
                    )
                ga = gpool.tile([P, mt], BF, tag="ga")
                nc.scalar.activation(ga[:], pg[:], gelu)
                nc.vector.tensor_mul(hT[:, h, :], ga[:], pv[:])

        def stage_b(ci):
            mt, tok0, ws = specs[ci]
            hT = hT_t.pop(ci)
            for t in range(mt // P):
                po = pso.tile([P, D], f32, tag="po")
                for h in range(NH):
                    nc.tensor.matmul(
                        out=po[:], lhsT=hT[:, h, t * P : (t + 1) * P],
                        rhs=wo_t[ws][h // 4][:, (h % 4) * D : (h % 4 + 1) * D],
                        start=(h == 0), stop=(h == NH - 1),
                    )
                ob = opool.tile([P, D], BF, tag="ob")
                j = tok0 // P + t
                nc.vector.tensor_scalar_mul(ob[:], po[:], wc_t[:, j : j + 1])
                nc.scalar.dma_start(out=out[j * P : (j + 1) * P, :], in_=ob[:])

        if len(specs) >= 2:
            # warm-up: first two chunks' stage A in two half-H phases so the
            # early PE stream only demands the first half of the A weights
            stage_a(0, 0, NH // 2)
            stage_a(1, 0, NH // 2)
            stage_a(0, NH // 2, NH)
            stage_a(1, NH // 2, NH)
            stage_b(0)
            stage_b(1)
            rest = range(2, len(specs))
        else:
            stage_a(0, 0, NH)
            stage_b(0)
            rest = range(0)
        for ci in rest:
            stage_a(ci, 0, NH)
            stage_b(ci)

    nc.compile()
    return nc


def _pack_weights(Wg_e, Wv_e, Wo_e):
    wg4 = Wg_e.reshape(ND, P, H).transpose(1, 0, 2)  # (P, ND, H)
    wv4 = Wv_e.reshape(ND, P, H).transpose(1, 0, 2)
    wgv = np.empty((NH // 2, P, 4 * D), BF16)
    for h in range(NH):
        hs = slice(h * P, (h + 1) * P)
        base = (h % 2) * 2 * D
        wgv[h // 2, :, base : base + D] = wg4[:, :, hs].reshape(P, D).astype(BF16)
        wgv[h // 2, :, base + D : base + 2 * D] = wv4[:, :, hs].reshape(P, D).astype(BF16)
    wo4 = np.ascontiguousarray(
        Wo_e.reshape(NH // 4, 4, P, D).transpose(0, 2, 1, 3).reshape(NH // 4, P, 4 * D)
    ).astype(BF16)
    return wgv, wo4


def _pack_x(x_rows, T, specs):
    """xp [P, ND*T] with per-chunk transposed d-major layout."""
    xT = np.zeros((D, T), np.float32)
    xT[:, : x_rows.shape[0]] = x_rows.T
    xp = np.empty((P, ND * T), BF16)
    for mt, tok0 in specs:
        blk = xT[:, tok0 : tok0 + mt].reshape(ND, P, mt).transpose(1, 0, 2)
        xp[:, ND * tok0 : ND * (tok0 + mt)] = blk.reshape(P, ND * mt).astype(BF16)
    return xp


def kernel(tokens, dispatch_weights, combine_weights, Wg, Wv, Wo, scales):
    from concourse.bass_utils import run_bass_kernel_spmd

    B, N, d_model = tokens.shape
    M = B * N
    x = np.ascontiguousarray(tokens.reshape(M, d_model), dtype=np.float32)
    disp = np.asarray(dispatch_weights).reshape(M, E)
    comb = np.asarray(combine_weights).reshape(M, E)
    w_all = np.where(disp > 0, comb, 0.0).astype(np.float32) * np.asarray(
        scales, np.float32
    )[None, :]

    idx = [np.nonzero(w_all[:, e])[0] for e in range(E)]
    counts = [len(i) for i in idx]
    CA, CB = _plan(counts, E)
    T = CA + CB
    specs = [(MT, i * MT) for i in range(CA // MT)]
    if CB:
        specs.append((CB, CA))

    # assign overflow slices (tokens beyond CA of hot experts) to cores' B slots
    b_assign = [None] * E  # per core: (expert, global token indices)
    if CB:
        free = list(range(E))
        for e in range(E):
            over = idx[e][CA:]
            for s in range(0, len(over), CB):
                b_assign[free.pop(0)] = (e, over[s : s + CB])

    if (CA, CB) not in _CACHE:
        _CACHE[(CA, CB)] = _build_program(CA, CB)
    nc = _CACHE[(CA, CB)]

    wpacked = [_pack_weights(
        np.asarray(Wg[e], np.float32), np.asarray(Wv[e], np.float32),
        np.asarray(Wo[e], np.float32)) for e in range(E)]

    in_maps = []
    for c in range(E):
        a_idx = idx[c][:CA]
        rows = [x[a_idx]]
        wrow = [w_all[a_idx, c]]
        if CB:
            pad_a = CA - len(a_idx)
            if pad_a:
                rows.append(np.zeros((pad_a, D), np.float32))
                wrow.append(np.zeros(pad_a, np.float32))
            if b_assign[c] is not None:
                be, b_idx = b_assign[c]
                rows.append(x[b_idx])
                wrow.append(w_all[b_idx, be])
            else:
                be = c
        x_rows = np.concatenate(rows, 0)
        w_tok = np.concatenate(wrow, 0)
        wcm = np.zeros((T // P, P), np.float32)
        wcm.reshape(-1)[: len(w_tok)] = w_tok
        m = {
            "xp": _pack_x(x_rows, T, specs),
            "wgva": wpacked[c][0],
            "woa": wpacked[c][1],
            "wc": np.ascontiguousarray(wcm.T),
        }
        if CB:
            m["wgvb"] = wpacked[be][0]
            m["wob"] = wpacked[be][1]
        in_maps.append(m)

    res = run_bass_kernel_spmd(nc, in_maps, list(range(E)))

    out = np.zeros((M, d_model), np.float32)
    for c in range(E):
        o = res.results[c]["out"]
        na = len(idx[c][:CA])
        out[idx[c][:CA]] += o[:na].astype(np.float32)
        if CB and b_assign[c] is not None:
            be, b_idx = b_assign[c]
            out[b_idx] += o[CA : CA + len(b_idx)].astype(np.float32)
    return out.reshape(B, N, d_model)
